# revision 6
# baseline (speedup 1.0000x reference)
import numpy as np
import concourse.bass as bass
import concourse.bacc as bacc_mod
import concourse.mybir as mybir
from concourse import tile
from concourse.bass_utils import run_bass_kernel_spmd

B, I, K, O, D = 128, 1152, 8, 32, 16
NC = 8
IL = I // NC          # 144 capsules per core
OD = O * D            # 512
CHA = 4               # phase-A i-chunk size
NCHA = IL // CHA      # 36
CH = 8                # route i-chunk size
NCH = IL // CH        # 18
NS1 = (I * K) // 128  # 72 s1-GEMM contraction chunks
EPS = 1e-8
NROUTES = 3
POOL_CHUNKS = set()   # (big mults all on DVE; Pool takes pair-adds)

F32 = mybir.dt.float32
BF16 = mybir.dt.bfloat16
ADD = mybir.AluOpType.add
MULT = mybir.AluOpType.mult
AF = mybir.ActivationFunctionType
AX = mybir.AxisListType
BOD = B + OD          # 640


def _build(reps=1):
    nc = bacc_mod.Bacc()
    wc_d = nc.declare_dram_parameter("wc", [NCHA, K, CHA * BOD], BF16,
                                     isOutput=False)
    s1_d = nc.declare_dram_parameter("s1ops", [NS1 // 2, 128, 2 * BOD], BF16,
                                     isOutput=False)
    id_d = nc.declare_dram_parameter("ident", [B, B], BF16, isOutput=False)
    v_d = nc.declare_dram_parameter("vout", [B, OD], BF16, isOutput=True)
    ar_in = [nc.dram_tensor(f"ar_in{r}", [B, OD], BF16)
             for r in range(2 * reps)]
    ar_out = [nc.dram_tensor(f"ar_out{r}", [B, OD], BF16)
              for r in range(2 * reps)]

    with tile.TileContext(nc) as tc:
        with (
            tc.tile_pool(name="big", bufs=1) as big,
            tc.tile_pool(name="ld", bufs=2) as ld,
            tc.tile_pool(name="lds", bufs=2) as lds,
            tc.tile_pool(name="ypool", bufs=2) as ypool,
            tc.tile_pool(name="sypool", bufs=2) as sypool,
            tc.tile_pool(name="pra", bufs=2) as pra,
            tc.tile_pool(name="sm", bufs=2) as sm,
            tc.tile_pool(name="sq", bufs=1) as sq,
            tc.tile_pool(name="small", bufs=1) as small,
            tc.tile_pool(name="ps_x", bufs=3, space="PSUM") as ps_x,
            tc.tile_pool(name="ps_z", bufs=2, space="PSUM") as ps_z,
            tc.tile_pool(name="ps_s", bufs=1, space="PSUM") as ps_s,
            tc.tile_pool(name="ps_1", bufs=1, space="PSUM") as ps_1,
        ):
            # persistent tiles
            xh = big.tile([B, IL * OD], BF16, tag="xh")      # 144KB/part
            ident = small.tile([B, B], BF16, tag="id")
            nc.sync.dma_start(out=ident[:], in_=id_d[:])
            zc = small.tile([B, 1], F32, tag="zc")
            nc.vector.memset(zc[:], 0.0)
            nc.const_aps.aps[(F32, 0.0)] = zc[:]
            ec = small.tile([B, 1], F32, tag="ec")
            nc.vector.memset(ec[:], EPS)
            nc.const_aps.aps[(F32, EPS)] = ec[:]
            u16 = small.tile([B, OD], BF16, tag="u16")
            u16b = small.tile([B, OD], BF16, tag="u16b")

            def squash_to(vdst32, s16):
                # s16: s [B, (d,o)] bf16; v = s * q/((1+q)sqrt(q+eps))
                t = sq.tile([B, OD], BF16, tag="sq_t")
                nc.vector.tensor_mul(t[:], s16[:], s16[:])
                q = sq.tile([B, O], F32, tag="qsq")
                nc.vector.tensor_reduce(
                    q[:], t[:].rearrange("p (d o) -> p o d", o=O),
                    axis=AX.X, op=ADD)
                r = sq.tile([B, O], F32, tag="rsq")
                nc.scalar.activation(r[:], q[:], AF.Sqrt, bias=EPS)
                t1 = sq.tile([B, O], F32, tag="t1sq")
                nc.vector.scalar_tensor_tensor(
                    t1[:], q[:], 1.0, r[:], op0=ADD, op1=MULT)
                t2 = sq.tile([B, O], F32, tag="t2sq")
                nc.vector.reciprocal(t2[:], t1[:])
                f = sq.tile([B, O], F32, tag="fsq")
                nc.vector.tensor_mul(f[:], q[:], t2[:])
                fb = f[:].rearrange("p (x o) -> p x o", x=1) \
                    .broadcast_to((B, D, O))
                nc.vector.tensor_mul(
                    vdst32[:].rearrange("p (d o) -> p d o", o=O),
                    s16[:].rearrange("p (d o) -> p d o", o=O), fb)

            for rep in range(reps):
                # ---------- phase A: x_hat (sharded) + s1 (replicated GEMM) --
                s1_ps = ps_1.tile([B, OD], F32, tag="s1ps")
                for ic in range(NCHA):
                    # s1-GEMM first: v1 is ready while x_hat still streams
                    s1_t = lds.tile([128, 2 * BOD], BF16, tag="s1t")
                    nc.sync.dma_start(out=s1_t[:], in_=s1_d[ic])
                    for h in range(2):
                        c = 2 * ic + h
                        nc.tensor.matmul(
                            s1_ps[:], s1_t[:, h * BOD:h * BOD + B],
                            s1_t[:, h * BOD + B:(h + 1) * BOD],
                            start=(c == 0), stop=(c == NS1 - 1))
                def xhat_chunk(ic):
                    w_t = ld.tile([K, CHA * BOD], BF16, tag="wt")
                    nc.sync.dma_start(out=w_t[:], in_=wc_d[ic])
                    for j in range(CHA):
                        i_g = ic * CHA + j
                        xh_ps = ps_x.tile([B, OD], F32, tag="xhps")
                        nc.tensor.matmul(
                            xh_ps[:], w_t[:, j * BOD:j * BOD + B],
                            w_t[:, j * BOD + B:(j + 1) * BOD],
                            start=True, stop=True)
                        dst = xh[:, i_g * OD:(i_g + 1) * OD]
                        if i_g % 2 == 0:
                            nc.vector.tensor_copy(dst, xh_ps[:])
                        else:
                            nc.scalar.copy(dst, xh_ps[:])

                # only the first two route-chunks' worth of x_hat upfront;
                # the rest streams under route 2 (PE fills route-2's DVE time)
                XPR = NCHA // NCH
                for ic in range(2 * XPR):
                    xhat_chunk(ic)

                # route 1: squash(s1/O) -> u2 = v1
                s16_1 = sq.tile([B, OD], BF16, tag="s16")
                nc.scalar.mul(s16_1[:], s1_ps[:], 1.0 / O)
                squash_to(u16, s16_1)

                # ---------- routes 2..3 (software-pipelined, skew 2) ---------
                for rt in range(2, NROUTES + 1):
                    last = rt == NROUTES
                    u_cur = u16 if rt == 2 else u16b
                    ub = u_cur[:].rearrange("p (x d o) -> p x d o",
                                            x=1, o=O) \
                        .broadcast_to((B, CH, D, O))
                    s_ps = ps_s.tile([B, OD], F32, tag="sps")
                    st = {}   # in-flight per-chunk state

                    def z_half(ic, st=st, ub=ub):
                        xs = xh[:, ic * CH * OD:(ic + 1) * CH * OD] \
                            .rearrange("p (i d o) -> p i d o", d=D, o=O)
                        y = ypool.tile([B, CH * OD], BF16, tag="y")
                        yv = y[:].rearrange("p (i d o) -> p i d o", d=D, o=O)
                        if ic in POOL_CHUNKS:
                            nc.gpsimd.tensor_mul(yv, xs, ub)
                        else:
                            nc.vector.tensor_mul(yv, xs, ub)
                        z_ps = ps_z.tile([B, CH * O], F32, tag="zps")
                        for h in range(2):
                            sl = slice(h * (CH // 2), (h + 1) * (CH // 2))
                            for d in range(D):
                                nc.tensor.matmul(
                                    z_ps[:, h * 128:(h + 1) * 128],
                                    ident[:], yv[:, sl, d, :],
                                    start=(d == 0), stop=(d == D - 1))
                        e = sm.tile([B, CH * O], BF16, tag="e")
                        nc.scalar.activation(e[:], z_ps[:], AF.Exp)
                        st[ic] = {"xs": xs, "y": y, "e": e}

                    def c_half(ic, st=st):
                        d_ = st[ic]
                        e = d_["e"]
                        sg = sm.tile([B, CH], F32, tag="sg")
                        nc.vector.tensor_reduce(
                            sg[:], e[:].rearrange("p (i o) -> p i o", o=O),
                            axis=AX.X, op=ADD)
                        rho = sm.tile([B, CH], F32, tag="rho")
                        nc.vector.reciprocal(rho[:], sg[:])
                        cb = sm.tile([B, CH * O], BF16, tag="cb")
                        rb = rho[:].rearrange("p (i x) -> p i x", x=1) \
                            .broadcast_to((B, CH, O))
                        nc.vector.tensor_mul(
                            cb[:].rearrange("p (i o) -> p i o", o=O),
                            e[:].rearrange("p (i o) -> p i o", o=O), rb)
                        sy = sypool.tile([B, CH * OD], BF16, tag="sy")
                        syv = sy[:].rearrange("p (i d o) -> p i d o", d=D, o=O)
                        cbb = cb[:].rearrange("p (i x o) -> p i x o",
                                              x=1, o=O) \
                            .broadcast_to((B, CH, D, O))
                        if ic in POOL_CHUNKS:
                            nc.gpsimd.tensor_mul(syv, d_["xs"], cbb)
                        else:
                            nc.vector.tensor_mul(syv, d_["xs"], cbb)
                        # pair-combine adjacent i
                        pr = pra.tile([B, (CH // 2) * OD], BF16, tag="pr")
                        sp = sy[:].rearrange("p (i2 two f) -> p i2 two f",
                                             two=2, f=OD)
                        for ph in range(2):
                            pp = slice(2 * ph, 2 * ph + 2)
                            nc.gpsimd.tensor_add(
                                pr[:, ph * OD * 2:(ph + 1) * OD * 2]
                                .rearrange("p (i2 f) -> p i2 f", f=OD),
                                sp[:, pp, 0, :], sp[:, pp, 1, :])
                        d_["pr"] = pr

                    def s_half(ic, st=st, s_ps=s_ps):
                        pr = st[ic]["pr"]
                        for jp in range(CH // 2):
                            gi = ic * (CH // 2) + jp
                            nc.tensor.matmul(
                                s_ps[:], ident[:],
                                pr[:, jp * OD:(jp + 1) * OD],
                                start=(gi == 0),
                                stop=(gi == NCH * (CH // 2) - 1))
                        del st[ic]

                    for t in range(NCH + 2):
                        if rt == 2 and t < NCH - 2:
                            for ic in range(XPR * (t + 2), XPR * (t + 3)):
                                xhat_chunk(ic)
                        if t < NCH:
                            z_half(t)
                        if 1 <= t <= NCH:
                            c_half(t - 1)
                        if t >= 2:
                            s_half(t - 2)

                    # AllReduce s (bf16)
                    k = 2 * rep + (rt - 2)
                    s16 = sq.tile([B, OD], BF16, tag="s16")
                    nc.scalar.copy(s16[:], s_ps[:])
                    nc.sync.dma_start(out=ar_in[k][:], in_=s16[:])
                    nc.gpsimd.collective_compute(
                        "AllReduce", ADD,
                        replica_groups=[list(range(NC))],
                        ins=[ar_in[k][:]], outs=[ar_out[k][:]])
                    sar16 = sq.tile([B, OD], BF16, tag="s16")
                    nc.sync.dma_start(out=sar16[:], in_=ar_out[k][:])
                    v16 = sq.tile([B, OD], BF16, tag="v16")
                    squash_to(v16, sar16)
                    if last:
                        nc.sync.dma_start(out=v_d[:], in_=v16[:])
                    else:
                        nc.vector.tensor_add(u16b[:], u16[:], v16[:])
    nc.compile()
    return nc


def _filter_bir(bir_json: bytes) -> bytes:
    """Drop same-ring WAW waits on DMAs (ring FIFO makes them redundant);
    the DIRECT2D descriptor only holds one wait command."""
    import json
    d = json.loads(bir_json)
    for fn in d.get("functions", []):
        for blk in fn.get("blocks", []):
            for inst in blk.get("instructions", []):
                if inst.get("opcode") != "DMACopy":
                    continue
                si = inst.get("sync_info") or {}
                waits = si.get("on_wait") or []
                if len(waits) <= 1:
                    continue
                ups = {u.get("ant_name") for u in (si.get("on_update") or [])}
                kept = [w for w in waits if w.get("ant_name") not in ups]
                if len(kept) < len(waits):
                    si["on_wait"] = kept
    return json.dumps(d).encode()


def _install_bir_filter():
    from concourse import bass2jax, bass_utils

    orig = bass_utils.compile_bir_kernel

    def patched(bir_json, tmpdir, neff_name="file.neff"):
        return orig(_filter_bir(bir_json), tmpdir, neff_name)

    bass2jax.compile_bir_kernel = patched


def _bf16(a: np.ndarray):
    import jax.numpy as jnp
    return np.asarray(jnp.asarray(a, dtype=jnp.bfloat16))


def _make_in_maps(x: np.ndarray, W: np.ndarray):
    ident = np.eye(B, dtype=np.float32)
    # replicated s1-GEMM operands: rows (i,k) global
    # x^T[(i,k), b] ; W'[(i,k), (d,o)] ; packed 2 chunks per DMA row-block
    xs1 = np.ascontiguousarray(x.transpose(1, 2, 0)).reshape(I * K, B)
    ws1 = np.ascontiguousarray(W.transpose(0, 2, 3, 1)).reshape(I * K, OD)
    s1flat = np.concatenate([xs1, ws1], axis=1).reshape(NS1, 128, BOD)
    s1ops = _bf16(np.ascontiguousarray(
        s1flat.reshape(NS1 // 2, 2, 128, BOD).transpose(0, 2, 1, 3)
        .reshape(NS1 // 2, 128, 2 * BOD)).astype(np.float32))
    id16 = _bf16(ident)
    in_maps = []
    for c in range(NC):
        sl = slice(c * IL, (c + 1) * IL)
        xt = np.ascontiguousarray(
            x[:, sl, :].transpose(2, 1, 0)).astype(np.float32)  # [K, IL, B]
        # W'[k, i, (d,o)] from W[i, o, k, d]
        wk = np.ascontiguousarray(
            W[sl].transpose(2, 0, 3, 1).reshape(K, IL, OD)).astype(np.float32)
        wc = np.concatenate([xt, wk], axis=2)  # [K, IL, B+OD]
        # chunk-major: [NCHA, K, CHA*BOD]
        wc_cm = np.ascontiguousarray(
            wc.reshape(K, NCHA, CHA, BOD).transpose(1, 0, 2, 3)
            .reshape(NCHA, K, CHA * BOD))
        in_maps.append({"wc": _bf16(wc_cm), "ident": id16, "s1ops": s1ops})
    return in_maps


def kernel(x: np.ndarray, W: np.ndarray) -> np.ndarray:
    _install_bir_filter()
    nc = _build()
    in_maps = _make_in_maps(x, W)
    res = run_bass_kernel_spmd(nc, in_maps, list(range(NC)))
    v = np.asarray(res.results[0]["vout"]).astype(np.float32)
    # device layout is [B, (d, o)] -> reference wants [B, O, D]
    return v.reshape(B, D, O).transpose(0, 2, 1)


if __name__ == "__main__":
    nc = _build()
    print("IR build OK")
